# revision 18
# baseline (speedup 1.0000x reference)
"""MoE block (RMSNorm + top-4 router + 32-expert GLU FFN) on 8 TRN2 NeuronCores.

Expert-parallel: core c owns experts [4c, 4c+4). Each core redundantly
computes the (tiny) RMSNorm + router over all 32 experts in f32, then runs a
dense masked FFN over all 64 tokens for its own 4 experts in fp16 (weights
host-cast; PSUM accumulation is f32), scaling each expert's contribution by
the routing weight (0 for unrouted tokens). gate_w/gate_b are passed to each
core with its own 4 experts permuted to rows 0..3, so the SPMD program
always reads routing columns 0..3 — no core-id branching.

The first FFN matmul keeps the token activations stationary on the PE and
streams w1 as the moving operand in 512-wide chunks (high MAC duty → PE
clocks up); h_act is transposed back to (I, T) on the PE for the second
matmul, which streams w2.

The host sums the 8 partial (T, D) outputs and adds the residual — that is
the "unshard" for expert parallelism.
"""

import sys
import types

sys.path.insert(0, "/opt/trn_rl_repo")

import numpy as np

D = 640
I = 640
E = 32
T = 64
K = 4
EPS = 1e-5
LIMIT = 7.0
BETA = 1.702
NCORES = 8
EPC = E // NCORES          # experts per core
NCH = D // 128             # 5 contraction chunks of 128

TRACE = False
PROF_DIR = None
LAST_EXEC_NS = None

_NC = None


def _ensure_ntff_hook():
    """boot() skips NTFF hook registration (image antenv lacks axon_hooks);
    provide the module so bass_utils can profile when TRACE=True."""
    if "antenv.axon_hooks" in sys.modules:
        return
    try:
        from trn_agent_boot.trn_boot import _ntff_profile_via_ctypes
        hook = _ntff_profile_via_ctypes("/opt/axon/libaxon_pjrt.so")
    except Exception:
        hook = None
    m = types.ModuleType("antenv.axon_hooks")
    m.get_axon_ntff_profile_hook = lambda: hook
    m.set_axon_ntff_profile_hook = lambda h: None
    sys.modules["antenv.axon_hooks"] = m


# h psum layout: (token, feature) tiles. glu = cols [0, I), lin = [I, 2I).
# big/small split keeps every psum tile within one 2KB bank (<=512 f32);
# the two 128-wide leftovers share one bank at free offsets 0/128.
H_SPECS = [("hgb", 2, 0, 512), ("hlb", 2, 640, 512), ("hsm", 1, None, 256)]


def _build():
    import concourse.bass as bass
    import concourse.bacc as bacc
    import concourse.tile as tile
    from concourse import mybir
    from concourse.masks import make_identity

    f32 = mybir.dt.float32
    f16 = mybir.dt.float16
    AF = mybir.ActivationFunctionType
    OP = mybir.AluOpType

    nc = bacc.Bacc("TRN2", target_bir_lowering=False, debug=False,
                   num_devices=NCORES)
    dx = nc.dram_tensor("x", (D, T), f32, kind="ExternalInput")
    dnw = nc.dram_tensor("norm_w", (D,), f32, kind="ExternalInput")
    dgw = nc.dram_tensor("gate_w", (E, D), f32, kind="ExternalInput")
    dgb = nc.dram_tensor("gate_b", (E,), f32, kind="ExternalInput")
    dw1 = nc.dram_tensor("w1", (EPC, D, 2 * I), f16, kind="ExternalInput")
    db1 = nc.dram_tensor("b1", (EPC, 2 * I), f16, kind="ExternalInput")
    dw2 = nc.dram_tensor("w2", (EPC, I, D), f16, kind="ExternalInput")
    db2 = nc.dram_tensor("b2", (EPC, D), f32, kind="ExternalInput")
    dout = nc.dram_tensor("out", (T, D), f32, kind="ExternalOutput")

    with tile.TileContext(nc) as tc:
        with (
            tc.tile_pool(name="consts", bufs=1) as consts,
            tc.tile_pool(name="small", bufs=2) as small,
            tc.tile_pool(name="wpool", bufs=3) as wpool,
            tc.tile_pool(name="hpool", bufs=2) as hpool,
            tc.tile_pool(name="ps_o", bufs=1, space="PSUM") as ps_o,
        ):
            # ---- earliest loads on the SP HWDGE ring: x and gate_w lead,
            # then the big expert-weight streams ----
            x_t = consts.tile([128, NCH, T], f32)
            nc.sync.dma_start(out=x_t,
                              in_=dx.ap().rearrange("(c p) t -> p c t", p=128))
            gwn = consts.tile([E, D], f32)
            nc.sync.dma_start(out=gwn, in_=dgw.ap())
            nw_t = consts.tile([128, NCH], f32)
            nc.sync.dma_start(out=nw_t,
                              in_=dnw.ap().rearrange("(c p) -> p c", p=128))
            gb_b = consts.tile([T, E], f32)
            gb_base = dgb.ap()
            nc.sync.dma_start(
                out=gb_b,
                in_=bass.AP(tensor=gb_base.tensor, offset=0,
                            ap=[[0, T], [1, E]]))
            b1_sb = consts.tile([1, EPC * 2 * I], f16)
            nc.sync.dma_start(out=b1_sb,
                              in_=db1.ap().rearrange("e i -> (e i)")[None, :])
            b2_t = consts.tile([EPC, D], f32)
            nc.sync.dma_start(out=b2_t, in_=db2.ap())
            w1r = dw1.ap().rearrange("e (c p) i -> e p c i", p=128)
            w2r = dw2.ap().rearrange("e (c p) i -> e p c i", p=128)
            w1_tiles, w2_tiles = [], []
            for e in range(EPC):
                w1_t = wpool.tile([128, NCH, 2 * I], f16, tag="w1")
                nc.sync.dma_start(out=w1_t, in_=w1r[e])
                w2_t = wpool.tile([128, NCH, D], f16, tag="w2")
                nc.sync.dma_start(out=w2_t, in_=w2r[e])
                w1_tiles.append(w1_t)
                w2_tiles.append(w2_t)

            ones128 = consts.tile([128, 128], f32)
            nc.vector.memset(ones128, 1.0)
            ones_hf = consts.tile([1, T], f16)
            nc.vector.memset(ones_hf, 1.0)
            eps_t = consts.tile([128, 1], f32)
            nc.vector.memset(eps_t, EPS)
            id64 = consts.tile([T, T], f32)
            make_identity(nc, id64)
            id_hf = consts.tile([T, T], f16)
            make_identity(nc, id_hf)

            with tc.tile_pool(name="ps_misc", bufs=2, space="PSUM") as ps_misc:
                # ---- RMSNorm (x is (D, T); D on partitions) ----
                xx = small.tile([128, NCH, T], f32, tag="xx")
                nc.vector.tensor_mul(xx, x_t, x_t)
                ps_ss = ps_misc.tile([128, T], f32, tag="misc")
                for c in range(NCH):
                    # ones.T @ xx chunk: broadcast sum over D to all parts
                    nc.tensor.matmul(ps_ss, ones128, xx[:, c, :],
                                     start=(c == 0), stop=(c == NCH - 1))
                sq = small.tile([128, T], f32, tag="sq")
                nc.scalar.activation(sq, ps_ss, AF.Sqrt, bias=eps_t,
                                     scale=1.0 / D)
                rstd = small.tile([128, T], f32, tag="rstd")
                nc.vector.reciprocal(rstd, sq)
                normed = consts.tile([128, NCH, T], f32)
                normed_hf = consts.tile([128, NCH, T], f16)
                for c in range(NCH):
                    xn = small.tile([128, T], f32, tag="xn")
                    nc.vector.tensor_scalar_mul(xn, x_t[:, c, :],
                                                nw_t[:, c:c + 1])
                    nc.vector.tensor_mul(normed[:, c, :], xn, rstd)
                    nc.vector.tensor_copy(normed_hf[:, c, :], normed[:, c, :])

                # gate_w.T (D on partitions) via PE transpose
                gwT = consts.tile([128, NCH, E], f32)
                for c in range(NCH):
                    ps_t = ps_misc.tile([128, E], f32, tag="misc")
                    nc.tensor.transpose(ps_t, gwn[:, 128 * c:128 * (c + 1)],
                                        id64[0:E, 0:E])
                    nc.scalar.copy(gwT[:, c, :], ps_t)

                # ---- router: gate, top-4, softmax, routing matrix A ----
                ps_g = ps_misc.tile([T, E], f32, tag="misc")
                for c in range(NCH):
                    nc.tensor.matmul(ps_g, normed[:, c, :], gwT[:, c, :],
                                     start=(c == 0), stop=(c == NCH - 1))
                g_sb = small.tile([T, E], f32, tag="g")
                nc.vector.tensor_add(g_sb, ps_g, gb_b)
                m8 = small.tile([T, 8], f32, tag="m8")
                nc.vector.max(m8, g_sb)
                negm = small.tile([T, 1], f32, tag="negm")
                nc.scalar.mul(negm, m8[:, 0:1], -1.0)
                s4 = small.tile([T, K], f32, tag="s4")
                nc.scalar.activation(s4, m8[:, 0:K], AF.Exp, bias=negm,
                                     scale=1.0)
                den = small.tile([T, 1], f32, tag="den")
                nc.vector.reduce_sum(den, s4, axis=mybir.AxisListType.X)
                rden = small.tile([T, 1], f32, tag="rden")
                nc.vector.reciprocal(rden, den)
                ew = small.tile([T, K], f32, tag="ew")
                nc.vector.tensor_scalar_mul(ew, s4, rden)

                A = small.tile([T, E], f32, tag="A")
                for k in range(K):
                    msk = small.tile([T, E], f32, tag="msk")
                    nc.vector.tensor_scalar(msk, g_sb, m8[:, k:k + 1], None,
                                            op0=OP.is_equal)
                    wm = small.tile([T, E], f32, tag="wm")
                    nc.vector.tensor_scalar_mul(wm, msk, ew[:, k:k + 1])
                    if k == 0:
                        nc.vector.tensor_copy(A, wm)
                    else:
                        nc.vector.tensor_add(A, A, wm)
                # h_act is computed as silu(beta*glu)*(lin+1) = beta * true
                # value; fold 1/beta into the per-expert routing scale.
                A_div = small.tile([T, E], f32, tag="A_div")
                nc.vector.tensor_scalar_mul(A_div, A, 1.0 / BETA)
                ps_a = ps_misc.tile([K, T], f32, tag="misc")
                nc.tensor.transpose(ps_a, A[:, 0:K], id64)
                a4t = small.tile([K, T], f32, tag="a4t")
                nc.scalar.copy(a4t, ps_a)

                # ---- bias-2 base: acc = A[:, :4] @ b2_shard ----
                acc = consts.tile([T, D], f32)
                ps_b1 = ps_o.tile([T, 512], f32, tag="o1")
                nc.tensor.matmul(ps_b1, a4t, b2_t[:, 0:512],
                                 start=True, stop=True)
                nc.scalar.copy(acc[:, 0:512], ps_b1)
                ps_b2 = ps_o.tile([T, 128], f32, tag="o2")
                nc.tensor.matmul(ps_b2, a4t, b2_t[:, 512:640],
                                 start=True, stop=True)
                nc.scalar.copy(acc[:, 512:640], ps_b2)

            # ---- experts: dense masked GLU FFN ----
            with (
                tc.tile_pool(name="ps_h", bufs=1, space="PSUM") as ps_h,
                tc.tile_pool(name="ps_tr", bufs=1, space="PSUM") as ps_tr,
            ):
                for e in range(EPC):
                    w1_t, w2_t = w1_tiles[e], w2_tiles[e]
                    hp = {}
                    for (tag, nbufs, ofs, n) in H_SPECS:
                        pt = ps_h.tile([T, n], f32, tag=tag, bufs=nbufs)
                        hp[tag] = pt
                        # one accumulation group per feature range: rank-1
                        # b1 bias first (resets psum), then the 5 d-chunks
                        ranges = ([(0, ofs, n)] if ofs is not None
                                  else [(0, 512, 128), (128, 1152, 128)])
                        for (po, fo, fn) in ranges:
                            nc.tensor.matmul(
                                pt[:, po:po + fn], ones_hf,
                                b1_sb[0:1, 2 * I * e + fo:2 * I * e + fo + fn],
                                start=True, stop=False)
                            for d in range(NCH):
                                nc.tensor.matmul(
                                    pt[:, po:po + fn], normed_hf[:, d, :],
                                    w1_t[:, d, fo:fo + fn],
                                    start=False, stop=(d == NCH - 1))
                    # activation: hact = silu(beta*min(glu,7))*(clip(lin)+1)
                    hact_b = hpool.tile([T, 512], f16, tag="hact_b")
                    hact_s = hpool.tile([T, 128], f16, tag="hact_s")
                    for (big, gl, ln, ha) in (
                        (512, hp["hgb"], hp["hlb"], hact_b),
                        (128, hp["hsm"][:, 0:128], hp["hsm"][:, 128:256],
                         hact_s),
                    ):
                        gm = hpool.tile([T, big], f32, tag=f"gm{big}")
                        nc.vector.tensor_scalar(gm, gl, LIMIT, None,
                                                op0=OP.min)
                        p_ = hpool.tile([T, big], f32, tag=f"p{big}")
                        nc.scalar.activation(p_, gm, AF.Silu, scale=BETA)
                        l1 = hpool.tile([T, big], f32, tag=f"l{big}")
                        nc.vector.tensor_scalar(l1, ln, LIMIT, None,
                                                op0=OP.min)
                        l2 = hpool.tile([T, big], f32, tag=f"l2{big}")
                        nc.vector.tensor_scalar(l2, l1, -LIMIT, 1.0,
                                                op0=OP.max, op1=OP.add)
                        nc.vector.tensor_mul(ha, p_, l2)
                    # transpose h_act to (I, T) and stream w2
                    ps_o1 = ps_o.tile([T, 512], f32, tag="o1")
                    ps_o2 = ps_o.tile([T, 128], f32, tag="o2")
                    for c in range(NCH):
                        src = (hact_b[:, 128 * c:128 * (c + 1)]
                               if c < 4 else hact_s)
                        pt = ps_tr.tile([128, T], f16, tag="tr")
                        nc.tensor.transpose(pt, src, id_hf)
                        hT = hpool.tile([128, T], f16, tag="hT", bufs=3)
                        nc.scalar.copy(hT, pt)
                        nc.tensor.matmul(ps_o1, hT, w2_t[:, c, 0:512],
                                         start=(c == 0), stop=(c == NCH - 1))
                        nc.tensor.matmul(ps_o2, hT, w2_t[:, c, 512:640],
                                         start=(c == 0), stop=(c == NCH - 1))
                    sc1 = small.tile([T, 512], f32, tag="sc1")
                    nc.vector.tensor_scalar_mul(sc1, ps_o1,
                                                A_div[:, e:e + 1])
                    nc.vector.tensor_add(acc[:, 0:512], acc[:, 0:512], sc1)
                    sc2 = small.tile([T, 128], f32, tag="sc2")
                    nc.vector.tensor_scalar_mul(sc2, ps_o2,
                                                A_div[:, e:e + 1])
                    nc.vector.tensor_add(acc[:, 512:640], acc[:, 512:640],
                                         sc2)

            nc.scalar.dma_start(out=dout.ap(), in_=acc)

    nc.finalize()
    return nc


def _get_nc():
    global _NC
    if _NC is None:
        _ensure_ntff_hook()
        _NC = _build()
    return _NC


def kernel(**inputs):
    global LAST_EXEC_NS
    nc = _get_nc()
    from concourse.bass_utils import run_bass_kernel_spmd

    x = np.ascontiguousarray(np.asarray(inputs["x"], dtype=np.float32))
    norm_w = np.ascontiguousarray(np.asarray(inputs["norm_w"], np.float32))
    gate_w = np.ascontiguousarray(np.asarray(inputs["gate_w"], np.float32))
    gate_b = np.ascontiguousarray(np.asarray(inputs["gate_b"], np.float32))
    w1 = np.asarray(inputs["w1"], np.float32).astype(np.float16)
    b1 = np.asarray(inputs["b1"], np.float32).astype(np.float16)
    w2 = np.asarray(inputs["w2"], np.float32).astype(np.float16)
    b2 = np.asarray(inputs["b2"], np.float32)

    x2 = np.ascontiguousarray(x[0, :, 0, :])  # (D, T)
    in_maps = []
    for c in range(NCORES):
        lo, hi = EPC * c, EPC * (c + 1)
        perm = np.r_[lo:hi, 0:lo, hi:E]
        in_maps.append({
            "x": x2,
            "norm_w": norm_w,
            "gate_w": np.ascontiguousarray(gate_w[perm]),
            "gate_b": np.ascontiguousarray(gate_b[perm]),
            "w1": np.ascontiguousarray(w1[lo:hi]),
            "b1": np.ascontiguousarray(b1[lo:hi]),
            "w2": np.ascontiguousarray(w2[lo:hi]),
            "b2": np.ascontiguousarray(b2[lo:hi]),
        })

    res = run_bass_kernel_spmd(nc, in_maps, core_ids=list(range(NCORES)),
                               trace=TRACE, tmpdir=PROF_DIR)
    LAST_EXEC_NS = res.exec_time_ns
    total = np.sum([r["out"] for r in res.results], axis=0)  # (T, D)
    return (x + total.T[None, :, None, :]).astype(np.float32)


# revision 19
# speedup vs baseline: 1.1321x; 1.1321x over previous
"""MoE block (RMSNorm + top-4 router + 32-expert GLU FFN) on 8 TRN2 NeuronCores.

Expert-parallel: core c owns experts [4c, 4c+4). Each core redundantly
computes the (tiny) RMSNorm + router over all 32 experts in f32, then runs a
dense masked FFN over all 64 tokens for its own 4 experts in fp16 (weights
host-cast; PSUM accumulation is f32), scaling each expert's contribution by
the routing weight (0 for unrouted tokens). gate_w/gate_b are passed to each
core with its own 4 experts permuted to rows 0..3, so the SPMD program
always reads routing columns 0..3 — no core-id branching.

The first FFN matmul keeps the token activations stationary on the PE and
streams w1 as the moving operand in 512-wide chunks (high MAC duty → PE
clocks up); h_act is transposed back to (I, T) on the PE for the second
matmul, which streams w2.

The host sums the 8 partial (T, D) outputs and adds the residual — that is
the "unshard" for expert parallelism.
"""

import sys
import types

sys.path.insert(0, "/opt/trn_rl_repo")

import numpy as np

D = 640
I = 640
E = 32
T = 64
K = 4
EPS = 1e-5
LIMIT = 7.0
BETA = 1.702
NCORES = 8
EPC = E // NCORES          # experts per core
NCH = D // 128             # 5 contraction chunks of 128

TRACE = False
PROF_DIR = None
LAST_EXEC_NS = None

_NC = None


def _ensure_ntff_hook():
    """boot() skips NTFF hook registration (image antenv lacks axon_hooks);
    provide the module so bass_utils can profile when TRACE=True."""
    if "antenv.axon_hooks" in sys.modules:
        return
    try:
        from trn_agent_boot.trn_boot import _ntff_profile_via_ctypes
        hook = _ntff_profile_via_ctypes("/opt/axon/libaxon_pjrt.so")
    except Exception:
        hook = None
    m = types.ModuleType("antenv.axon_hooks")
    m.get_axon_ntff_profile_hook = lambda: hook
    m.set_axon_ntff_profile_hook = lambda h: None
    sys.modules["antenv.axon_hooks"] = m


# h psum layout: (token, feature) tiles. glu = cols [0, I), lin = [I, 2I).
# big/small split keeps every psum tile within one 2KB bank (<=512 f32);
# the two 128-wide leftovers share one bank at free offsets 0/128.
H_SPECS = [("hgb", 2, 0, 512), ("hlb", 2, 640, 512), ("hsm", 1, None, 256)]


def _build():
    import concourse.bass as bass
    import concourse.bacc as bacc
    import concourse.tile as tile
    from concourse import mybir
    from concourse.masks import make_identity

    f32 = mybir.dt.float32
    f16 = mybir.dt.float16
    AF = mybir.ActivationFunctionType
    OP = mybir.AluOpType

    nc = bacc.Bacc("TRN2", target_bir_lowering=False, debug=False,
                   num_devices=NCORES)
    dx = nc.dram_tensor("x", (D, T), f32, kind="ExternalInput")
    dnw = nc.dram_tensor("norm_w", (D,), f32, kind="ExternalInput")
    dgw = nc.dram_tensor("gate_w", (E, D), f32, kind="ExternalInput")
    dgb = nc.dram_tensor("gate_b", (E,), f32, kind="ExternalInput")
    dw1 = nc.dram_tensor("w1", (EPC, D, 2 * I), f16, kind="ExternalInput")
    db1 = nc.dram_tensor("b1", (EPC, 2 * I), f16, kind="ExternalInput")
    dw2 = nc.dram_tensor("w2", (EPC, I, D), f16, kind="ExternalInput")
    db2 = nc.dram_tensor("b2", (EPC, D), f32, kind="ExternalInput")
    dout = nc.dram_tensor("out", (T, D), f32, kind="ExternalOutput")

    with tile.TileContext(nc) as tc:
        with (
            tc.tile_pool(name="consts", bufs=1) as consts,
            tc.tile_pool(name="small", bufs=2) as small,
            tc.tile_pool(name="wpool", bufs=3) as wpool,
            tc.tile_pool(name="hpool", bufs=2) as hpool,
            tc.tile_pool(name="ps_o", bufs=1, space="PSUM") as ps_o,
        ):
            # ---- earliest loads on the SP HWDGE ring: x and gate_w lead,
            # then the big expert-weight streams ----
            x_t = consts.tile([128, NCH, T], f32)
            nc.sync.dma_start(out=x_t,
                              in_=dx.ap().rearrange("(c p) t -> p c t", p=128))
            gwn = consts.tile([E, D], f32)
            nc.sync.dma_start(out=gwn, in_=dgw.ap())
            b1_sb = consts.tile([1, EPC * 2 * I], f16)
            nc.gpsimd.dma_start(out=b1_sb,
                                in_=db1.ap().rearrange("e i -> (e i)")[None, :])
            nw_t = consts.tile([128, NCH], f32)
            nc.gpsimd.dma_start(out=nw_t,
                                in_=dnw.ap().rearrange("(c p) -> p c", p=128))
            gb_b = consts.tile([T, E], f32)
            gb_base = dgb.ap()
            nc.gpsimd.dma_start(
                out=gb_b,
                in_=bass.AP(tensor=gb_base.tensor, offset=0,
                            ap=[[0, T], [1, E]]))
            b2_t = consts.tile([EPC, D], f32)
            nc.gpsimd.dma_start(out=b2_t, in_=db2.ap())
            w1r = dw1.ap().rearrange("e (c p) i -> e p c i", p=128)
            w2r = dw2.ap().rearrange("e (c p) i -> e p c i", p=128)
            w1_tiles, w2_tiles = [], []
            for e in range(EPC):
                w1_t = wpool.tile([128, NCH, 2 * I], f16, tag="w1")
                nc.sync.dma_start(out=w1_t, in_=w1r[e])
                w2_t = wpool.tile([128, NCH, D], f16, tag="w2")
                nc.sync.dma_start(out=w2_t, in_=w2r[e])
                w1_tiles.append(w1_t)
                w2_tiles.append(w2_t)

            ones128 = consts.tile([128, 128], f32)
            nc.vector.memset(ones128, 1.0)
            ones_hf = consts.tile([1, T], f16)
            nc.vector.memset(ones_hf, 1.0)
            eps_t = consts.tile([128, 1], f32)
            nc.vector.memset(eps_t, EPS)
            id64 = consts.tile([T, T], f32)
            make_identity(nc, id64)
            id_hf = consts.tile([T, T], f16)
            make_identity(nc, id_hf)

            with tc.tile_pool(name="ps_misc", bufs=2, space="PSUM") as ps_misc:
                # ---- RMSNorm (x is (D, T); D on partitions) ----
                xx = small.tile([128, NCH, T], f32, tag="xx")
                nc.vector.tensor_mul(xx, x_t, x_t)
                ps_ss = ps_misc.tile([128, T], f32, tag="misc")
                for c in range(NCH):
                    # ones.T @ xx chunk: broadcast sum over D to all parts
                    nc.tensor.matmul(ps_ss, ones128, xx[:, c, :],
                                     start=(c == 0), stop=(c == NCH - 1))
                sq = small.tile([128, T], f32, tag="sq")
                nc.scalar.activation(sq, ps_ss, AF.Sqrt, bias=eps_t,
                                     scale=1.0 / D)
                rstd = small.tile([128, T], f32, tag="rstd")
                nc.vector.reciprocal(rstd, sq)
                normed = consts.tile([128, NCH, T], f32)
                normed_hf = consts.tile([128, NCH, T], f16)
                for c in range(NCH):
                    xn = small.tile([128, T], f32, tag="xn")
                    nc.vector.tensor_scalar_mul(xn, x_t[:, c, :],
                                                nw_t[:, c:c + 1])
                    nc.vector.tensor_mul(normed[:, c, :], xn, rstd)
                    nc.vector.tensor_copy(normed_hf[:, c, :], normed[:, c, :])

                # gate_w.T (D on partitions) via PE transpose
                gwT = consts.tile([128, NCH, E], f32)
                for c in range(NCH):
                    ps_t = ps_misc.tile([128, E], f32, tag="misc")
                    nc.tensor.transpose(ps_t, gwn[:, 128 * c:128 * (c + 1)],
                                        id64[0:E, 0:E])
                    nc.scalar.copy(gwT[:, c, :], ps_t)

                # ---- router: gate, top-4, softmax, routing matrix A ----
                ps_g = ps_misc.tile([T, E], f32, tag="misc")
                for c in range(NCH):
                    nc.tensor.matmul(ps_g, normed[:, c, :], gwT[:, c, :],
                                     start=(c == 0), stop=(c == NCH - 1))
                g_sb = small.tile([T, E], f32, tag="g")
                nc.vector.tensor_add(g_sb, ps_g, gb_b)
                m8 = small.tile([T, 8], f32, tag="m8")
                nc.vector.max(m8, g_sb)
                negm = small.tile([T, 1], f32, tag="negm")
                nc.scalar.mul(negm, m8[:, 0:1], -1.0)
                s4 = small.tile([T, K], f32, tag="s4")
                nc.scalar.activation(s4, m8[:, 0:K], AF.Exp, bias=negm,
                                     scale=1.0)
                den = small.tile([T, 1], f32, tag="den")
                nc.vector.reduce_sum(den, s4, axis=mybir.AxisListType.X)
                rden = small.tile([T, 1], f32, tag="rden")
                nc.vector.reciprocal(rden, den)
                ew = small.tile([T, K], f32, tag="ew")
                nc.vector.tensor_scalar_mul(ew, s4, rden)

                A = small.tile([T, E], f32, tag="A")
                for k in range(K):
                    msk = small.tile([T, E], f32, tag="msk")
                    nc.vector.tensor_scalar(msk, g_sb, m8[:, k:k + 1], None,
                                            op0=OP.is_equal)
                    wm = small.tile([T, E], f32, tag="wm")
                    nc.vector.tensor_scalar_mul(wm, msk, ew[:, k:k + 1])
                    if k == 0:
                        nc.vector.tensor_copy(A, wm)
                    else:
                        nc.vector.tensor_add(A, A, wm)
                # h_act is computed as silu(beta*glu)*(lin+1) = beta * true
                # value; fold 1/beta into the per-expert routing scale.
                A_div = small.tile([T, E], f32, tag="A_div")
                nc.vector.tensor_scalar_mul(A_div, A, 1.0 / BETA)
                ps_a = ps_misc.tile([K, T], f32, tag="misc")
                nc.tensor.transpose(ps_a, A[:, 0:K], id64)
                a4t = small.tile([K, T], f32, tag="a4t")
                nc.scalar.copy(a4t, ps_a)

                # ---- bias-2 base: acc = A[:, :4] @ b2_shard ----
                acc = consts.tile([T, D], f32)
                ps_b1 = ps_o.tile([T, 512], f32, tag="o1")
                nc.tensor.matmul(ps_b1, a4t, b2_t[:, 0:512],
                                 start=True, stop=True)
                nc.scalar.copy(acc[:, 0:512], ps_b1)
                ps_b2 = ps_o.tile([T, 128], f32, tag="o2")
                nc.tensor.matmul(ps_b2, a4t, b2_t[:, 512:640],
                                 start=True, stop=True)
                nc.scalar.copy(acc[:, 512:640], ps_b2)

            # ---- experts: dense masked GLU FFN ----
            with (
                tc.tile_pool(name="ps_h", bufs=1, space="PSUM") as ps_h,
                tc.tile_pool(name="ps_tr", bufs=1, space="PSUM") as ps_tr,
            ):
                for e in range(EPC):
                    w1_t, w2_t = w1_tiles[e], w2_tiles[e]
                    hp = {}
                    for (tag, nbufs, ofs, n) in H_SPECS:
                        pt = ps_h.tile([T, n], f32, tag=tag, bufs=nbufs)
                        hp[tag] = pt
                        # one accumulation group per feature range: rank-1
                        # b1 bias first (resets psum), then the 5 d-chunks
                        ranges = ([(0, ofs, n)] if ofs is not None
                                  else [(0, 512, 128), (128, 1152, 128)])
                        for (po, fo, fn) in ranges:
                            nc.tensor.matmul(
                                pt[:, po:po + fn], ones_hf,
                                b1_sb[0:1, 2 * I * e + fo:2 * I * e + fo + fn],
                                start=True, stop=False)
                            for d in range(NCH):
                                nc.tensor.matmul(
                                    pt[:, po:po + fn], normed_hf[:, d, :],
                                    w1_t[:, d, fo:fo + fn],
                                    start=False, stop=(d == NCH - 1))
                    # activation: hact = silu(beta*min(glu,7))*(clip(lin)+1)
                    hact_b = hpool.tile([T, 512], f16, tag="hact_b")
                    hact_s = hpool.tile([T, 128], f16, tag="hact_s")
                    for (big, gl, ln, ha) in (
                        (512, hp["hgb"], hp["hlb"], hact_b),
                        (128, hp["hsm"][:, 0:128], hp["hsm"][:, 128:256],
                         hact_s),
                    ):
                        gm = hpool.tile([T, big], f32, tag=f"gm{big}")
                        nc.vector.tensor_scalar(gm, gl, LIMIT, None,
                                                op0=OP.min)
                        p_ = hpool.tile([T, big], f32, tag=f"p{big}")
                        nc.scalar.activation(p_, gm, AF.Silu, scale=BETA)
                        l1 = hpool.tile([T, big], f32, tag=f"l{big}")
                        nc.vector.tensor_scalar(l1, ln, LIMIT, None,
                                                op0=OP.min)
                        l2 = hpool.tile([T, big], f32, tag=f"l2{big}")
                        nc.vector.tensor_scalar(l2, l1, -LIMIT, 1.0,
                                                op0=OP.max, op1=OP.add)
                        nc.vector.tensor_mul(ha, p_, l2)
                    # transpose h_act to (I, T) and stream w2
                    ps_o1 = ps_o.tile([T, 512], f32, tag="o1")
                    ps_o2 = ps_o.tile([T, 128], f32, tag="o2")
                    for c in range(NCH):
                        src = (hact_b[:, 128 * c:128 * (c + 1)]
                               if c < 4 else hact_s)
                        pt = ps_tr.tile([128, T], f16, tag="tr")
                        nc.tensor.transpose(pt, src, id_hf)
                        hT = hpool.tile([128, T], f16, tag="hT", bufs=3)
                        nc.scalar.copy(hT, pt)
                        nc.tensor.matmul(ps_o1, hT, w2_t[:, c, 0:512],
                                         start=(c == 0), stop=(c == NCH - 1))
                        nc.tensor.matmul(ps_o2, hT, w2_t[:, c, 512:640],
                                         start=(c == 0), stop=(c == NCH - 1))
                    sc1 = small.tile([T, 512], f32, tag="sc1")
                    nc.vector.tensor_scalar_mul(sc1, ps_o1,
                                                A_div[:, e:e + 1])
                    nc.vector.tensor_add(acc[:, 0:512], acc[:, 0:512], sc1)
                    sc2 = small.tile([T, 128], f32, tag="sc2")
                    nc.vector.tensor_scalar_mul(sc2, ps_o2,
                                                A_div[:, e:e + 1])
                    nc.vector.tensor_add(acc[:, 512:640], acc[:, 512:640],
                                         sc2)

            nc.scalar.dma_start(out=dout.ap(), in_=acc)

    nc.finalize()
    return nc


def _get_nc():
    global _NC
    if _NC is None:
        _ensure_ntff_hook()
        _NC = _build()
    return _NC


def kernel(**inputs):
    global LAST_EXEC_NS
    nc = _get_nc()
    from concourse.bass_utils import run_bass_kernel_spmd

    x = np.ascontiguousarray(np.asarray(inputs["x"], dtype=np.float32))
    norm_w = np.ascontiguousarray(np.asarray(inputs["norm_w"], np.float32))
    gate_w = np.ascontiguousarray(np.asarray(inputs["gate_w"], np.float32))
    gate_b = np.ascontiguousarray(np.asarray(inputs["gate_b"], np.float32))
    w1 = np.asarray(inputs["w1"], np.float32).astype(np.float16)
    b1 = np.asarray(inputs["b1"], np.float32).astype(np.float16)
    w2 = np.asarray(inputs["w2"], np.float32).astype(np.float16)
    b2 = np.asarray(inputs["b2"], np.float32)

    x2 = np.ascontiguousarray(x[0, :, 0, :])  # (D, T)
    in_maps = []
    for c in range(NCORES):
        lo, hi = EPC * c, EPC * (c + 1)
        perm = np.r_[lo:hi, 0:lo, hi:E]
        in_maps.append({
            "x": x2,
            "norm_w": norm_w,
            "gate_w": np.ascontiguousarray(gate_w[perm]),
            "gate_b": np.ascontiguousarray(gate_b[perm]),
            "w1": np.ascontiguousarray(w1[lo:hi]),
            "b1": np.ascontiguousarray(b1[lo:hi]),
            "w2": np.ascontiguousarray(w2[lo:hi]),
            "b2": np.ascontiguousarray(b2[lo:hi]),
        })

    res = run_bass_kernel_spmd(nc, in_maps, core_ids=list(range(NCORES)),
                               trace=TRACE, tmpdir=PROF_DIR)
    LAST_EXEC_NS = res.exec_time_ns
    total = np.sum([r["out"] for r in res.results], axis=0)  # (T, D)
    return (x + total.T[None, :, None, :]).astype(np.float32)
